# revision 34
# baseline (speedup 1.0000x reference)
"""Positional-encoding kernel for Trainium2 (8 NeuronCores, SPMD).

Computes out = x + pos_embedding[pos] where pos[i] is the segment-local
index of row i (batch is sorted segment ids).

Host re-lays rows into 128-partition tiles so every on-device add is a
static slice against an SBUF-resident block table:

  * head tiles: 128 consecutive rows of one graph at local position
    128*b -> add table block b over all 128 partitions.
  * tail pieces: the last (<128) rows of a graph, cut into 32-row pieces
    at local position 128*bt + 32*m.  Pieces of equal key (bt, m) are
    packed 4 per tile; the augmented table block for (bt, m) holds the
    32 embedding rows replicated across the four partition bands, so a
    whole tail tile is still a single full-partition add.

Slots are sorted by table-block key, so a run of consecutive slots
shares one block and becomes ONE tensor_tensor add with a stride-0
(broadcast) source AP -- compute instruction count stays tiny.

Everything runs in the quantized domain x' = x / SCALE (SCALE chosen so
|x' + e'| < 127); x ships as int8 and all output returns as int8, so
HBM traffic is 2 B/elem total.  Measured per-FD-elem engine rates (ns):
DVE int8 add 1.06, DVE bf16 add 0.54 (2x mode), ACT convert 0.87.
DMA engines bill by WRITE-side bytes (~400 GB/s aggregate over 16
engines), so a casting store (bf16 SBUF -> int8 HBM) costs the same
1 B/elem as a plain int8 store while eliminating the out-convert.

Casting DMAs bill at their LARGER side (2 B/elem), so all transfers
stay int8 and dtype logistics run on engines.  Two streams balance
DVE / ACT / PE under the ~71 us/core DMA floor:

  * sA (62.5%): int8 -> DVE mixed add (int8 + bf16 table -> int8, 1x)
    in place -> plain int8 store.           DVE 1.06/elem.
  * sP (37.5%): int8 -> ACT convert to bf16 -> TensorE identity-matmul
    pair (psum = I@x + I@e) -> ACT evac PSUM -> int8 SBUF (doubles as
    the out-convert) -> plain int8 store.   ACT 1.83 + PE 2.2/elem.
    (Accumulating a matmul onto PSUM content written by ACT corrupts
    on HW -- PSUM accumulation only sees prior MATMUL writes -- so
    both matmuls are required.  An sC variant that keeps the add on
    DVE in bf16 raises aggregate ACT work and measured worse; fa far
    above 0.625 degrades the DVE rate and leaves a DVE-only tail.)

This lands DVE ~71us, ACT ~73us, PE ~86us, DMA ~71us per core.
(GpSimd tensor ops are avoided entirely: measured 2.4-4.2 ns/elem AND
they stall concurrent DVE ops 4-7x.)  Worst-case |err| ~ 1.1*SCALE ~
1% of max|out|, inside the 2e-2 tolerance.  Units of each key are
dealt round-robin across the 8 cores with counts padded to equal ->
every core runs the *same* static SPMD program.
"""

import numpy as np

NCORES = 8
P = 128          # partitions / tile rows
BAND = 32        # tail piece granularity (compute partition-range quantum)
CHUNKSA = (24, 16, 8, 4, 2, 1)    # int8-stream chunk sizes (tiles)
CHUNKSP = (16, 8, 4, 2, 1)        # PE-stream chunk sizes (tiles)
CHUNKSC = (8, 4, 2, 1)            # bf16/DVE-stream chunk sizes (tiles)
RAMPA = (4, 8)       # warm-up chunks so the first adds start early
RAMPP = (4, 8)
RAMPC = ()
# slot -> stream pattern, repeated: 0=sA, 1=sP, 2=sC.  5/3 A/P:
# aggregate engine-work per elem is A 1.06 (DVE), P ~4 (ACT+PE),
# C 2.28 (ACT+DVE); fa much above 0.625 degrades the DVE rate
# (SBUF contention) and leaves a DVE-only tail -- measured worse.
STREAM_PAT = (0, 1, 0, 0, 1, 0, 1, 0)

_prog_cache = {}


def _chunks_of(T, sizes, ramp=()):
    """ascending warm-up ramp + big-first greedy (ends small naturally)."""
    out = []
    rem = T
    for r in ramp:
        if rem >= r + sizes[0]:
            out.append(r)
            rem -= r
    for s in sizes:
        while rem >= s:
            out.append(s)
            rem -= s
    assert rem == 0
    return out


def _build_program(TA, TP, TC, NB, H, keysA, keysP, keysC):
    """keys*[slot] = block index into the augmented table (sorted runs)."""
    import concourse.tile as tile
    from concourse import bacc, mybir

    nc = bacc.Bacc("TRN2", target_bir_lowering=False, debug=False)
    bf16 = mybir.dt.bfloat16
    f32 = mybir.dt.float32
    i8 = mybir.dt.int8
    xa_t = nc.dram_tensor("xa", [P, max(TA, 1) * H], i8,
                          kind="ExternalInput").ap()
    xp_t = nc.dram_tensor("xp", [P, max(TP, 1) * H], i8,
                          kind="ExternalInput").ap()
    xc_t = oc_t = None
    if TC > 0:
        xc_t = nc.dram_tensor("xc", [P, TC * H], i8,
                              kind="ExternalInput").ap()
        oc_t = nc.dram_tensor("outc", [P, TC * H], i8,
                              kind="ExternalOutput").ap()
    e_t = nc.dram_tensor("etab", [P, NB * H], bf16, kind="ExternalInput").ap()
    id_t = nc.dram_tensor("ident", [P, P], bf16, kind="ExternalInput").ap()
    oa_t = nc.dram_tensor("outa", [P, max(TA, 1) * H], i8,
                          kind="ExternalOutput").ap()
    op_t = nc.dram_tensor("outp", [P, max(TP, 1) * H], i8,
                          kind="ExternalOutput").ap()

    # chunk schedule: interleave the streams by progress so all engines
    # and both DMA directions stay busy throughout
    cl = [_chunks_of(TA, CHUNKSA, RAMPA),
          _chunks_of(TP, CHUNKSP, RAMPP),
          _chunks_of(TC, CHUNKSC, RAMPC)]
    tot = [max(TA, 1), max(TP, 1), max(TC, 1)]
    ix = [0, 0, 0]
    done = [0, 0, 0]
    plan = []       # (stream, base, ct)
    while any(ix[s] < len(cl[s]) for s in range(3)):
        # bias: keep the P/C streams (longer per-chunk latency chains)
        # a little ahead so all streams finish together
        s = min((s for s in range(3) if ix[s] < len(cl[s])),
                key=lambda s: done[s] / tot[s] + (0.05 if s == 0 else 0))
        plan.append((s, done[s], cl[s][ix[s]]))
        done[s] += cl[s][ix[s]]
        ix[s] += 1

    xs_t = {1: xp_t, 2: xc_t}
    os_t = {1: op_t, 2: oc_t}

    with tile.TileContext(nc) as tc:
        with (
            tc.tile_pool(name="const", bufs=1) as cpool,
            tc.tile_pool(name="wa", bufs=4) as wpoolA,
            tc.tile_pool(name="w8", bufs=3) as wpool8,
            tc.tile_pool(name="wb", bufs=3) as wpoolB,
            tc.tile_pool(name="wo", bufs=3) as wpoolO,
            tc.tile_pool(name="ps", bufs=2, space="PSUM") as pspool,
        ):
            et = cpool.tile([P, NB * H], bf16)
            ident = cpool.tile([P, P], bf16)
            # table loads ride the (initially idle) ACT queue; block 0
            # lands first so the earliest adds only wait ~0.3us
            nc.scalar.dma_start(ident[:], id_t)
            nc.scalar.dma_start(et[:, 0:H], e_t[:, 0:H])
            if NB > 1:
                nc.scalar.dma_start(et[:, H:], e_t[:, H:])

            def add_runs(t, keys, base, ct):
                u = 0
                while u < ct:
                    c = keys[base + u]
                    L = 1
                    while u + L < ct and keys[base + u + L] == c:
                        L += 1
                    dst = t[:, u * H:(u + L) * H].rearrange(
                        "p (l h) -> p l h", h=H)
                    src = et[:, c * H:(c + 1) * H][:, None, :].to_broadcast(
                        (P, L, H))
                    nc.vector.tensor_add(dst, dst, src)
                    u += L

            # P/C chunk in-DMA runs one chunk AHEAD of the body; the ACT
            # converts are emitted per 4-tile GROUP, one group ahead of
            # that group's matmuls, so on the ACT queue short convs and
            # evacs alternate -- PSUM buffers recycle every ~2 ACT ops
            # instead of stalling PE behind a whole-chunk convert
            bchunks = [(st, base, ct) for (st, base, ct) in plan if st != 0]
            bt8 = {}
            btb = {}

            def b_front(j):
                st, base, ct = bchunks[j]
                t8 = wpool8.tile([P, ct * H], i8, tag="w8")
                nc.sync.dma_start(t8[:], xs_t[st][:, base * H:(base + ct) * H])
                bt8[j] = t8
                tb = wpoolB.tile([P, ct * H], bf16, tag="wb")
                btb[j] = tb

            bgroups = []    # (chunk j, g0, gn)
            for j, (st, base, ct) in enumerate(bchunks):
                for g0 in range(0, ct, 4):
                    bgroups.append((j, g0, min(4, ct - g0)))

            def b_conv(q):
                j, g0, gn = bgroups[q]
                nc.scalar.copy(btb[j][:, g0 * H:(g0 + gn) * H],
                               bt8[j][:, g0 * H:(g0 + gn) * H])

            # out-DMAs ride the gpsimd (SWDGE) queue, emitted DELAY
            # chunks late: by then their producer's semaphore is already
            # set, so the out's sem-wait never blocks a later trigger
            # behind it on the same queue (head-of-line)
            DELAY = 2
            pend = []
            jb = 0          # next P/C chunk body to emit
            nbf = 0         # P/C chunk dmas emitted so far
            qb = 0          # next group body to emit
            qc = 0          # group convs emitted so far
            for stream, base, ct in plan:
                if stream == 0:
                    t = wpoolA.tile([P, ct * H], i8, tag="wa")
                    nc.sync.dma_start(t[:], xa_t[:, base * H:(base + ct) * H])
                    add_runs(t, keysA, base, ct)
                    pend.append((oa_t[:, base * H:(base + ct) * H], t))
                else:
                    if nbf == jb:
                        b_front(nbf)
                        nbf += 1
                    if nbf < len(bchunks):
                        b_front(nbf)
                        nbf += 1
                    tb = btb[jb]
                    to = wpoolO.tile([P, ct * H], i8, tag="wo")
                    if stream == 1:
                        # TensorE adds: psum = I@x + I@e per tile; ACT
                        # evacuates 4 tiles of PSUM at once straight to
                        # int8 (the evac IS the out-convert)
                        for g0 in range(0, ct, 4):
                            gn = min(4, ct - g0)
                            while qc <= qb:
                                b_conv(qc)
                                qc += 1
                            if qc < len(bgroups) and \
                                    bgroups[qc][0] <= jb + 1:
                                b_conv(qc)
                                qc += 1
                            qb += 1
                            ps = pspool.tile([P, gn * H], f32, tag="ps")
                            for i in range(gn):
                                u = g0 + i
                                c = keysP[base + u]
                                nc.tensor.matmul(
                                    ps[:, i * H:(i + 1) * H], ident[:],
                                    tb[:, u * H:(u + 1) * H],
                                    start=True, stop=False)
                                nc.tensor.matmul(
                                    ps[:, i * H:(i + 1) * H], ident[:],
                                    et[:, c * H:(c + 1) * H],
                                    start=False, stop=True)
                            nc.scalar.copy(to[:, g0 * H:(g0 + gn) * H],
                                           ps[:])
                    else:
                        while qc <= qb or (qc < len(bgroups) and
                                           bgroups[qc][0] <= jb):
                            b_conv(qc)
                            qc += 1
                        qb += -(-ct // 4)
                        add_runs(tb, keysC, base, ct)     # DVE bf16 2x
                        nc.scalar.copy(to[:], tb[:])      # ACT bf16->int8
                    jb += 1
                    pend.append((os_t[stream][:, base * H:(base + ct) * H],
                                 to))
                if len(pend) > DELAY:
                    dst, src = pend.pop(0)
                    nc.gpsimd.dma_start(dst, src[:])
            for dst, src in pend:
                nc.gpsimd.dma_start(dst, src[:])
    nc.compile()
    return nc


def _plan(batch, N):
    """Returns (keys, blocks, units) where keys[slot] = table block per
    slot (same for all cores), blocks = list of block descriptors
    ("h", b) or ("t", bt, m), and units[k] = list of
    (slot, band_lo, src_row, nrows) row-range placements for core k."""
    change = np.flatnonzero(batch[1:] != batch[:-1]) + 1
    starts = np.concatenate([[0], change]).astype(np.int64)
    ends = np.concatenate([change, [N]]).astype(np.int64)
    lens = ends - starts

    head_byb = {}   # b -> [graph start rows]
    tail_bykey = {}  # (bt, m) -> [(abs start row, nrows)]
    for s, L in zip(starts.tolist(), lens.tolist()):
        nb = L // P
        for b in range(nb):
            head_byb.setdefault(b, []).append(s + b * P)
        r = L % P
        if r:
            for m in range((r + BAND - 1) // BAND):
                tail_bykey.setdefault((nb, m), []).append(
                    (s + nb * P + BAND * m, min(BAND, r - BAND * m)))

    blocks = [("h", b) for b in sorted(head_byb)]
    blkid = {("h", b): i for i, (_, b) in enumerate(blocks)}
    for key in sorted(tail_bykey):
        blkid[("t",) + key] = len(blocks)
        blocks.append(("t",) + key)

    keys = []
    units = [[] for _ in range(NCORES)]
    slot = 0
    for b in sorted(head_byb):
        lst = head_byb[b]
        per = -(-len(lst) // NCORES)
        lst = lst + [-1] * (per * NCORES - len(lst))
        for i in range(per):
            for k in range(NCORES):
                s = lst[i * NCORES + k]
                if s >= 0:
                    units[k].append((slot + i, 0, s, P))
        keys.extend([blkid[("h", b)]] * per)
        slot += per

    for key in sorted(tail_bykey):
        lst = tail_bykey[key]
        per = -(-len(lst) // NCORES)          # pieces per core
        tiles = -(-per // 4)
        per = tiles * 4
        lst = lst + [None] * (per * NCORES - len(lst))
        for i in range(per):
            for k in range(NCORES):
                pc = lst[i * NCORES + k]
                if pc is not None:
                    units[k].append(
                        (slot + i // 4, BAND * (i % 4), pc[0], pc[1]))
        keys.extend([blkid[("t",) + key]] * tiles)
        slot += tiles

    return keys, blocks, units, slot


def kernel(x, batch, pos_embedding):
    import ml_dtypes
    from concourse.bass_utils import run_bass_kernel_spmd

    x = np.ascontiguousarray(np.asarray(x, dtype=np.float32))
    batch = np.asarray(batch).astype(np.int64).ravel()
    E = np.ascontiguousarray(np.asarray(pos_embedding, dtype=np.float32))
    N, H = x.shape

    keys, blocks, units, T = _plan(batch, N)
    NB = len(blocks)

    # stream split; every key sub-list stays sorted, so runs stay long
    pat = np.asarray(STREAM_PAT)
    sid = pat[np.arange(T) % len(pat)]
    gslot = np.empty(T, dtype=np.int64)       # global slot -> local slot
    for s in range(3):
        m = sid == s
        gslot[m] = np.arange(int(m.sum()))
    keys = np.asarray(keys)
    keysA = keys[sid == 0].tolist()
    keysP = keys[sid == 1].tolist()
    keysC = keys[sid == 2].tolist()
    TA, TP, TC = len(keysA), len(keysP), len(keysC)

    # quantization: x' = x/s, table carries e/s; |x' + e'| < 127
    scale = max((np.abs(x).max() + np.abs(E).max()) / 126.0, 1e-30)
    x_q = np.rint(x * (1.0 / scale)).astype(np.int8)

    # augmented table, partition-major: block ("h", b)[p] = E[128b + p];
    # block ("t", bt, m)[p] = E[128bt + 32m + (p % 32)]
    etab = np.empty((P, NB * H), dtype=np.float32)
    parange = np.arange(P)
    for c, blk in enumerate(blocks):
        if blk[0] == "h":
            rows = blk[1] * P + parange
        else:
            rows = blk[1] * P + BAND * blk[2] + (parange % BAND)
        etab[:, c * H:(c + 1) * H] = E[rows]
    etab = (etab * (1.0 / scale)).astype(ml_dtypes.bfloat16)

    idxs = [np.full((NCORES, P, max(t, 1)), -1, dtype=np.int64)
            for t in (TA, TP, TC)]
    for k in range(NCORES):
        for slot, p0, src, n in units[k]:
            idxs[sid[slot]][k, p0:p0 + n, gslot[slot]] = \
                np.arange(src, src + n)
    valids = [ix >= 0 for ix in idxs]

    x_devs = [np.ascontiguousarray(
        x_q[np.where(valids[s], idxs[s], 0)].reshape(NCORES, P, -1))
        for s in range(3)]

    pkey = (TA, TP, TC, NB, H, tuple(keysA), tuple(keysP), tuple(keysC))
    nc = _prog_cache.get(pkey)
    if nc is None:
        nc = _build_program(TA, TP, TC, NB, H, keysA, keysP, keysC)
        _prog_cache.clear()
        _prog_cache[pkey] = nc

    ident = np.eye(P, dtype=np.float32).astype(ml_dtypes.bfloat16)
    in_maps = []
    for k in range(NCORES):
        m = {"xa": x_devs[0][k], "xp": x_devs[1][k],
             "etab": etab, "ident": ident}
        if TC > 0:
            m["xc"] = x_devs[2][k]
        in_maps.append(m)
    res = run_bass_kernel_spmd(nc, in_maps, core_ids=list(range(NCORES)),
                               trace=kernel._trace)
    kernel._last_exec_ns = res.exec_time_ns

    out = np.empty_like(x)
    for k in range(NCORES):
        for s, oname, t in ((0, "outa", TA), (1, "outp", TP),
                            (2, "outc", TC)):
            if t == 0:
                continue
            o = np.asarray(res.results[k][oname]).reshape(P, -1, H)
            m = valids[s][k]
            out[idxs[s][k][m]] = o[m].astype(np.float32) * scale
    return out


kernel._trace = False
kernel._last_exec_ns = None


# revision 37
# speedup vs baseline: 1.1874x; 1.1874x over previous
"""Positional-encoding kernel for Trainium2 (8 NeuronCores, SPMD).

Computes out = x + pos_embedding[pos] where pos[i] is the segment-local
index of row i (batch is sorted segment ids).

Host re-lays rows into 128-partition tiles so every on-device add is a
static slice against an SBUF-resident block table:

  * head tiles: 128 consecutive rows of one graph at local position
    128*b -> add table block b over all 128 partitions.
  * tail pieces: the last (<128) rows of a graph, cut into 32-row pieces
    at local position 128*bt + 32*m.  Pieces of equal key (bt, m) are
    packed 4 per tile; the augmented table block for (bt, m) holds the
    32 embedding rows replicated across the four partition bands, so a
    whole tail tile is still a single full-partition add.

Slots are sorted by table-block key, so a run of consecutive slots
shares one block and becomes ONE tensor_tensor add with a stride-0
(broadcast) source AP -- compute instruction count stays tiny.

Everything runs in the quantized domain x' = x / SCALE (SCALE chosen so
|x' + e'| < 127); x ships as int8 and all output returns as int8, so
HBM traffic is 2 B/elem total.  Measured per-FD-elem engine rates (ns):
DVE int8 add 1.06, DVE bf16 add 0.54 (2x mode), ACT convert 0.87.
DMA engines bill by WRITE-side bytes (~400 GB/s aggregate over 16
engines), so a casting store (bf16 SBUF -> int8 HBM) costs the same
1 B/elem as a plain int8 store while eliminating the out-convert.

Casting DMAs bill at their LARGER side (2 B/elem), so all transfers
stay int8 and dtype logistics run on engines.  Two streams balance
DVE / ACT / PE under the ~71 us/core DMA floor:

  * sA (62.5%): int8 -> DVE mixed add (int8 + bf16 table -> int8, 1x)
    in place -> plain int8 store.           DVE 1.06/elem.
  * sP (37.5%): int8 -> ACT convert to bf16 -> TensorE identity-matmul
    pair (psum = I@x + I@e) -> ACT evac PSUM -> int8 SBUF (doubles as
    the out-convert) -> plain int8 store.   ACT 1.83 + PE 2.2/elem.
    (Accumulating a matmul onto PSUM content written by ACT corrupts
    on HW -- PSUM accumulation only sees prior MATMUL writes -- so
    both matmuls are required.  An sC variant that keeps the add on
    DVE in bf16 raises aggregate ACT work and measured worse; fa far
    above 0.625 degrades the DVE rate and leaves a DVE-only tail.)

This lands DVE ~71us, ACT ~73us, PE ~86us, DMA ~71us per core.
(GpSimd tensor ops are avoided entirely: measured 2.4-4.2 ns/elem AND
they stall concurrent DVE ops 4-7x.)  Worst-case |err| ~ 1.1*SCALE ~
1% of max|out|, inside the 2e-2 tolerance.  Units of each key are
dealt round-robin across the 8 cores with counts padded to equal ->
every core runs the *same* static SPMD program.
"""

import numpy as np

NCORES = 8
P = 128          # partitions / tile rows
BAND = 32        # tail piece granularity (compute partition-range quantum)
CHUNKSA = (24, 16, 8, 4, 2, 1)    # int8-stream chunk sizes (tiles)
CHUNKSP = (8, 4, 2, 1)            # PE-stream chunk sizes (tiles)
CHUNKSC = (8, 4, 2, 1)            # bf16/DVE-stream chunk sizes (tiles)
RAMPA = (4, 8)       # warm-up chunks so the first adds start early
RAMPP = (4, 8)
RAMPC = ()
# slot -> stream pattern, repeated: 0=sA, 1=sP, 2=sC.  5/3 A/P:
# aggregate engine-work per elem is A 1.06 (DVE), P ~4 (ACT+PE),
# C 2.28 (ACT+DVE); fa much above 0.625 degrades the DVE rate
# (SBUF contention) and leaves a DVE-only tail -- measured worse.
STREAM_PAT = (0, 1, 0, 0, 1, 0, 1, 0)

_prog_cache = {}


def _chunks_of(T, sizes, ramp=()):
    """ascending warm-up ramp + big-first greedy (ends small naturally)."""
    out = []
    rem = T
    for r in ramp:
        if rem >= r + sizes[0]:
            out.append(r)
            rem -= r
    for s in sizes:
        while rem >= s:
            out.append(s)
            rem -= s
    assert rem == 0
    return out


def _build_program(TA, TP, TC, NB, H, keysA, keysP, keysC):
    """keys*[slot] = block index into the augmented table (sorted runs)."""
    import concourse.tile as tile
    from concourse import bacc, mybir

    nc = bacc.Bacc("TRN2", target_bir_lowering=False, debug=False)
    bf16 = mybir.dt.bfloat16
    f32 = mybir.dt.float32
    i8 = mybir.dt.int8
    xa_t = nc.dram_tensor("xa", [P, max(TA, 1) * H], i8,
                          kind="ExternalInput").ap()
    xp_t = nc.dram_tensor("xp", [P, max(TP, 1) * H], i8,
                          kind="ExternalInput").ap()
    xc_t = oc_t = None
    if TC > 0:
        xc_t = nc.dram_tensor("xc", [P, TC * H], i8,
                              kind="ExternalInput").ap()
        oc_t = nc.dram_tensor("outc", [P, TC * H], i8,
                              kind="ExternalOutput").ap()
    e_t = nc.dram_tensor("etab", [P, NB * H], bf16, kind="ExternalInput").ap()
    id_t = nc.dram_tensor("ident", [P, P], bf16, kind="ExternalInput").ap()
    oa_t = nc.dram_tensor("outa", [P, max(TA, 1) * H], i8,
                          kind="ExternalOutput").ap()
    op_t = nc.dram_tensor("outp", [P, max(TP, 1) * H], i8,
                          kind="ExternalOutput").ap()

    # chunk schedule: interleave the streams by progress so all engines
    # and both DMA directions stay busy throughout
    cl = [_chunks_of(TA, CHUNKSA, RAMPA),
          _chunks_of(TP, CHUNKSP, RAMPP),
          _chunks_of(TC, CHUNKSC, RAMPC)]
    tot = [max(TA, 1), max(TP, 1), max(TC, 1)]
    ix = [0, 0, 0]
    done = [0, 0, 0]
    plan = []       # (stream, base, ct)
    while any(ix[s] < len(cl[s]) for s in range(3)):
        # bias: keep the P/C streams (longer per-chunk latency chains)
        # a little ahead so all streams finish together
        s = min((s for s in range(3) if ix[s] < len(cl[s])),
                key=lambda s: done[s] / tot[s] + (0.05 if s == 0 else 0))
        plan.append((s, done[s], cl[s][ix[s]]))
        done[s] += cl[s][ix[s]]
        ix[s] += 1

    xs_t = {1: xp_t, 2: xc_t}
    os_t = {1: op_t, 2: oc_t}

    with tile.TileContext(nc) as tc:
        with (
            tc.tile_pool(name="const", bufs=1) as cpool,
            tc.tile_pool(name="wa", bufs=4) as wpoolA,
            tc.tile_pool(name="w8", bufs=3) as wpool8,
            tc.tile_pool(name="wb", bufs=3) as wpoolB,
            tc.tile_pool(name="wo", bufs=3) as wpoolO,
            tc.tile_pool(name="ps", bufs=2, space="PSUM") as pspool,
        ):
            et = cpool.tile([P, NB * H], bf16)
            ident = cpool.tile([P, P], bf16)
            # table loads ride the (initially idle) ACT queue; block 0
            # lands first so the earliest adds only wait ~0.3us
            nc.scalar.dma_start(ident[:], id_t)
            nc.scalar.dma_start(et[:, 0:H], e_t[:, 0:H])
            if NB > 1:
                nc.scalar.dma_start(et[:, H:], e_t[:, H:])

            def add_runs(t, keys, base, ct):
                u = 0
                while u < ct:
                    c = keys[base + u]
                    L = 1
                    while u + L < ct and keys[base + u + L] == c:
                        L += 1
                    dst = t[:, u * H:(u + L) * H].rearrange(
                        "p (l h) -> p l h", h=H)
                    src = et[:, c * H:(c + 1) * H][:, None, :].to_broadcast(
                        (P, L, H))
                    nc.vector.tensor_add(dst, dst, src)
                    u += L

            # P/C chunk in-DMA + ACT conv run one chunk AHEAD of the
            # chunk's body: the ACT queue then orders conv(k+1) BEFORE
            # evacs/out-convs(k), so PE/DVE never stall on a convert
            # queued behind them.  P chunks are kept small so a convert
            # occupies the ACT queue only ~3.6us at a time and PSUM
            # recycling evacs slot in promptly.
            bchunks = [(st, base, ct) for (st, base, ct) in plan if st != 0]
            btb = {}

            def b_front(j):
                st, base, ct = bchunks[j]
                t8 = wpool8.tile([P, ct * H], i8, tag="w8")
                nc.sync.dma_start(t8[:], xs_t[st][:, base * H:(base + ct) * H])
                tb = wpoolB.tile([P, ct * H], bf16, tag="wb")
                nc.scalar.copy(tb[:], t8[:])   # ACT int8 -> bf16 (exact)
                btb[j] = tb

            # out-DMAs ride the gpsimd (SWDGE) queue, emitted DELAY
            # chunks late: by then their producer's semaphore is already
            # set, so the out's sem-wait never blocks a later trigger
            # behind it on the same queue (head-of-line)
            DELAY = 2
            pend = []
            jb = 0          # next P/C chunk body to emit
            nbf = 0         # P/C fronts emitted so far
            for stream, base, ct in plan:
                if stream == 0:
                    t = wpoolA.tile([P, ct * H], i8, tag="wa")
                    nc.sync.dma_start(t[:], xa_t[:, base * H:(base + ct) * H])
                    add_runs(t, keysA, base, ct)
                    pend.append((oa_t[:, base * H:(base + ct) * H], t))
                else:
                    if nbf == jb:
                        b_front(nbf)
                        nbf += 1
                    if nbf < len(bchunks):
                        b_front(nbf)
                        nbf += 1
                    tb = btb.pop(jb)
                    jb += 1
                    to = wpoolO.tile([P, ct * H], i8, tag="wo")
                    if stream == 1:
                        # TensorE adds: psum = I@x + I@e per tile; ACT
                        # evacuates 4 tiles of PSUM at once straight to
                        # int8 (the evac IS the out-convert)
                        for g0 in range(0, ct, 4):
                            gn = min(4, ct - g0)
                            ps = pspool.tile([P, gn * H], f32, tag="ps")
                            for i in range(gn):
                                u = g0 + i
                                c = keysP[base + u]
                                nc.tensor.matmul(
                                    ps[:, i * H:(i + 1) * H], ident[:],
                                    tb[:, u * H:(u + 1) * H],
                                    start=True, stop=False)
                                nc.tensor.matmul(
                                    ps[:, i * H:(i + 1) * H], ident[:],
                                    et[:, c * H:(c + 1) * H],
                                    start=False, stop=True)
                            nc.scalar.copy(to[:, g0 * H:(g0 + gn) * H],
                                           ps[:])
                    else:
                        add_runs(tb, keysC, base, ct)     # DVE bf16 2x
                        nc.scalar.copy(to[:], tb[:])      # ACT bf16->int8
                    pend.append((os_t[stream][:, base * H:(base + ct) * H],
                                 to))
                if len(pend) > DELAY:
                    dst, src = pend.pop(0)
                    nc.gpsimd.dma_start(dst, src[:])
            for dst, src in pend:
                nc.gpsimd.dma_start(dst, src[:])
    nc.compile()
    return nc


def _plan(batch, N):
    """Returns (keys, blocks, units) where keys[slot] = table block per
    slot (same for all cores), blocks = list of block descriptors
    ("h", b) or ("t", bt, m), and units[k] = list of
    (slot, band_lo, src_row, nrows) row-range placements for core k."""
    change = np.flatnonzero(batch[1:] != batch[:-1]) + 1
    starts = np.concatenate([[0], change]).astype(np.int64)
    ends = np.concatenate([change, [N]]).astype(np.int64)
    lens = ends - starts

    head_byb = {}   # b -> [graph start rows]
    tail_bykey = {}  # (bt, m) -> [(abs start row, nrows)]
    for s, L in zip(starts.tolist(), lens.tolist()):
        nb = L // P
        for b in range(nb):
            head_byb.setdefault(b, []).append(s + b * P)
        r = L % P
        if r:
            for m in range((r + BAND - 1) // BAND):
                tail_bykey.setdefault((nb, m), []).append(
                    (s + nb * P + BAND * m, min(BAND, r - BAND * m)))

    blocks = [("h", b) for b in sorted(head_byb)]
    blkid = {("h", b): i for i, (_, b) in enumerate(blocks)}
    for key in sorted(tail_bykey):
        blkid[("t",) + key] = len(blocks)
        blocks.append(("t",) + key)

    keys = []
    units = [[] for _ in range(NCORES)]
    slot = 0
    for b in sorted(head_byb):
        lst = head_byb[b]
        per = -(-len(lst) // NCORES)
        lst = lst + [-1] * (per * NCORES - len(lst))
        for i in range(per):
            for k in range(NCORES):
                s = lst[i * NCORES + k]
                if s >= 0:
                    units[k].append((slot + i, 0, s, P))
        keys.extend([blkid[("h", b)]] * per)
        slot += per

    for key in sorted(tail_bykey):
        lst = tail_bykey[key]
        per = -(-len(lst) // NCORES)          # pieces per core
        tiles = -(-per // 4)
        per = tiles * 4
        lst = lst + [None] * (per * NCORES - len(lst))
        for i in range(per):
            for k in range(NCORES):
                pc = lst[i * NCORES + k]
                if pc is not None:
                    units[k].append(
                        (slot + i // 4, BAND * (i % 4), pc[0], pc[1]))
        keys.extend([blkid[("t",) + key]] * tiles)
        slot += tiles

    return keys, blocks, units, slot


def kernel(x, batch, pos_embedding):
    import ml_dtypes
    from concourse.bass_utils import run_bass_kernel_spmd

    x = np.ascontiguousarray(np.asarray(x, dtype=np.float32))
    batch = np.asarray(batch).astype(np.int64).ravel()
    E = np.ascontiguousarray(np.asarray(pos_embedding, dtype=np.float32))
    N, H = x.shape

    keys, blocks, units, T = _plan(batch, N)
    NB = len(blocks)

    # stream split; every key sub-list stays sorted, so runs stay long
    pat = np.asarray(STREAM_PAT)
    sid = pat[np.arange(T) % len(pat)]
    gslot = np.empty(T, dtype=np.int64)       # global slot -> local slot
    for s in range(3):
        m = sid == s
        gslot[m] = np.arange(int(m.sum()))
    keys = np.asarray(keys)
    keysA = keys[sid == 0].tolist()
    keysP = keys[sid == 1].tolist()
    keysC = keys[sid == 2].tolist()
    TA, TP, TC = len(keysA), len(keysP), len(keysC)

    # quantization: x' = x/s, table carries e/s; |x' + e'| < 127
    scale = max((np.abs(x).max() + np.abs(E).max()) / 126.0, 1e-30)
    x_q = np.rint(x * (1.0 / scale)).astype(np.int8)

    # augmented table, partition-major: block ("h", b)[p] = E[128b + p];
    # block ("t", bt, m)[p] = E[128bt + 32m + (p % 32)]
    etab = np.empty((P, NB * H), dtype=np.float32)
    parange = np.arange(P)
    for c, blk in enumerate(blocks):
        if blk[0] == "h":
            rows = blk[1] * P + parange
        else:
            rows = blk[1] * P + BAND * blk[2] + (parange % BAND)
        etab[:, c * H:(c + 1) * H] = E[rows]
    etab = (etab * (1.0 / scale)).astype(ml_dtypes.bfloat16)

    idxs = [np.full((NCORES, P, max(t, 1)), -1, dtype=np.int64)
            for t in (TA, TP, TC)]
    for k in range(NCORES):
        for slot, p0, src, n in units[k]:
            idxs[sid[slot]][k, p0:p0 + n, gslot[slot]] = \
                np.arange(src, src + n)
    valids = [ix >= 0 for ix in idxs]

    x_devs = [np.ascontiguousarray(
        x_q[np.where(valids[s], idxs[s], 0)].reshape(NCORES, P, -1))
        for s in range(3)]

    pkey = (TA, TP, TC, NB, H, tuple(keysA), tuple(keysP), tuple(keysC))
    nc = _prog_cache.get(pkey)
    if nc is None:
        nc = _build_program(TA, TP, TC, NB, H, keysA, keysP, keysC)
        _prog_cache.clear()
        _prog_cache[pkey] = nc

    ident = np.eye(P, dtype=np.float32).astype(ml_dtypes.bfloat16)
    in_maps = []
    for k in range(NCORES):
        m = {"xa": x_devs[0][k], "xp": x_devs[1][k],
             "etab": etab, "ident": ident}
        if TC > 0:
            m["xc"] = x_devs[2][k]
        in_maps.append(m)
    res = run_bass_kernel_spmd(nc, in_maps, core_ids=list(range(NCORES)),
                               trace=kernel._trace)
    kernel._last_exec_ns = res.exec_time_ns

    out = np.empty_like(x)
    for k in range(NCORES):
        for s, oname, t in ((0, "outa", TA), (1, "outp", TP),
                            (2, "outc", TC)):
            if t == 0:
                continue
            o = np.asarray(res.results[k][oname]).reshape(P, -1, H)
            m = valids[s][k]
            out[idxs[s][k][m]] = o[m].astype(np.float32) * scale
    return out


kernel._trace = False
kernel._last_exec_ns = None
